# revision 1
# baseline (speedup 1.0000x reference)
"""GAT (3-layer graph attention + final linear) Trainium2 Bass kernel.

Problem: B=4 graphs, N=2048 atoms, D=128, H=256.
  per layer: h = relu(x @ W.T + b); e_ij = leaky_relu(f1_i + f2_j, 0.01)
  masked by adj; att = softmax_j(e); x = x + att @ h.
  final: relu(x @ Wt.T + bt).

Sharding: 8 cores; core c -> (graph b=c//2, row-half s=c%2 of the NxN
attention). Each core computes full h for its graph (cheap), attention
only for its 1024 rows. Between layers the updated x rows are exchanged
within (2b, 2b+1) pairs via AllGather.

Key tricks:
  - additive mask maskT[j,i] = 9e15*adj[i,j] - 9e15 precomputed once
    (transposed via PE, stored bf16), kept in SBUF; each layer a PE
    identity-matmul writes it into PSUM (start=True) and one K=2 outer
    product [f2;1]x[1;f1] accumulates the logits on top, so masking
    costs no separate NxN pass (and no DVE time).
  - softmax without row-max: logits bounded (~36) so exp is safe in f32;
    validated vs reference (rel err 3.6e-7).
  - exp(leaky(z)) = max(exp(z), exp(0.01*z)): two ACT Exp passes (same
    LUT set, no table reloads) + one DVE max.
  - attention aggregated transposed (attoutT = hnat[j]^T @ p, one
    stationary load per j-tile, 512-wide bf16 streams); row sums via a
    ones-column matmul; normalization happens after transposing back,
    on [NS,D] instead of an NxN pass.

Hardware wait-slot discipline (walrus limits: DMA instr = 1 sem wait,
engine instr = 2): every DMA is a first-write to its slot or its slot
was last touched by a single full-coverage engine write (gpsimd memset);
small weights are "laundered" through DVE copies so no matmul depends on
two DMA queues.
"""

import numpy as np

import concourse.bass as bass
import concourse.mybir as mybir
import concourse.tile as tile
from concourse import masks
from concourse.bass_utils import run_bass_kernel_spmd

P = 128
F32 = mybir.dt.float32
BF16 = mybir.dt.bfloat16
I32 = mybir.dt.int32
AF = mybir.ActivationFunctionType
OP = mybir.AluOpType

NEGC = 9e15


def _legalize_waits(nc, dma_limit=1, engine_limit=1):
    """Walrus can encode only 1 sem wait on a DMA instruction and ~2 on an
    engine instruction. Move excess waits onto standalone EventSemaphore
    instructions (1 wait each) inserted just before the offender on the
    same engine."""
    counter = [0]

    def split(ins):
        si = ins.sync_info
        if si is None:
            return None
        limit = dma_limit if type(ins).__name__.startswith("InstDMA") \
            else engine_limit
        waits = list(si.on_wait)
        if len(waits) <= limit:
            return None
        keep = waits[-limit:] if limit > 0 else []
        extra = waits[:-limit] if limit > 0 else waits
        evs = []
        for w in extra:
            counter[0] += 1
            evs.append(mybir.InstEventSemaphore(
                name=f"evsplit{counter[0]}", engine=ins.engine,
                sync_info=mybir.SyncInfo(on_wait=[w], on_update=[])))
        ins.sync_info = mybir.SyncInfo(on_wait=keep,
                                       on_update=list(si.on_update))
        return evs

    for f in nc.m.functions:
        for blk in f.blocks:
            new_list = []
            changed = False
            for ins in blk.instructions:
                evs = split(ins)
                if evs:
                    new_list.extend(evs)
                    changed = True
                new_list.append(ins)
            if changed:
                blk.instructions = new_list


def build_gat_nc(N, NS, D, H, num_cores, pair_groups, nlayers=3,
                 legalize=True):
    assert D == P and NS % 512 == 0 and N % 512 == 0
    nj = N // P        # j tiles
    nit = NS // P      # i tiles in shard
    nch = NS // 512    # 512-chunks in shard
    nchN = N // 512
    nH = H // P

    nc = bass.Bass("TRN2", target_bir_lowering=False, debug=False,
                   num_devices=num_cores)

    # ---- I/O ----
    xT_in = nc.dram_tensor("xT", [P, N], F32, kind="ExternalInput")
    xTs_in = nc.dram_tensor("xTs", [P, NS], F32, kind="ExternalInput")
    xs_in = nc.dram_tensor("xs", [NS, D], F32, kind="ExternalInput")
    adj_in = nc.dram_tensor("adj_s", [NS, N], I32, kind="ExternalInput")
    WT_in = [nc.dram_tensor(f"WT{l}", [D, D], F32, kind="ExternalInput")
             for l in range(nlayers)]
    bv_in = [nc.dram_tensor(f"bv{l}", [D, 1], F32, kind="ExternalInput")
             for l in range(nlayers)]
    av_in = [nc.dram_tensor(f"av{l}", [D, 2], F32, kind="ExternalInput")
             for l in range(nlayers)]
    WtT_in = nc.dram_tensor("WtT", [D, H], F32, kind="ExternalInput")
    btp_in = nc.dram_tensor("btp", [P, nH], F32, kind="ExternalInput")
    out_ext = nc.dram_tensor("out_s", [NS, H], F32, kind="ExternalOutput")

    # DRAM bounce buffers for the pair AllGather of xT shards
    ag_in = [nc.dram_tensor(f"ag_in{l}", [P, NS], F32)
             for l in range(nlayers - 1)]
    ag_out = [nc.dram_tensor(f"ag_out{l}", [2 * P, NS], F32)
              for l in range(nlayers - 1)]

    with tile.TileContext(nc) as tc:
        import contextlib
        ctx = contextlib.ExitStack()
        with ctx:
            persist = ctx.enter_context(tc.tile_pool(name="persist", bufs=1))
            rawp = ctx.enter_context(tc.tile_pool(name="rawp", bufs=2))
            convp = ctx.enter_context(tc.tile_pool(name="convp", bufs=4))
            qp = ctx.enter_context(tc.tile_pool(name="qp", bufs=2))
            xsp = ctx.enter_context(tc.tile_pool(name="xsp", bufs=2))
            xtp = ctx.enter_context(tc.tile_pool(name="xtp", bufs=2))
            smallp = ctx.enter_context(tc.tile_pool(name="smallp", bufs=4))
            ocp = ctx.enter_context(tc.tile_pool(name="ocp", bufs=2))
            pe_pool = ctx.enter_context(
                tc.tile_pool(name="pe_pool", bufs=2, space="PSUM"))
            attp = ctx.enter_context(
                tc.tile_pool(name="attp", bufs=1, space="PSUM"))
            spp = ctx.enter_context(
                tc.tile_pool(name="spp", bufs=1, space="PSUM"))

            _bp = [(pe_pool, "pe"), (pe_pool, "pe"), (attp, "att"),
                   (spp, "s")]
            _bi = [0]

            def btile(shape, dt, name):
                pool, tag = _bp[_bi[0] % 4]
                _bi[0] += 1
                return pool.tile(shape, dt, name=name, tag=tag)

            ident = persist.tile([P, P], F32)
            masks.make_identity(nc, ident[:])
            identb = persist.tile([P, P], BF16)
            masks.make_identity(nc, identb[:])

            # ---- persistent state ----
            maskT = [persist.tile([P, NS], BF16, name=f"maskT{j}",
                                  tag=f"maskT{j}") for j in range(nj)]
            hT = persist.tile([P, N], F32)
            hsT = persist.tile([P, NS], F32)
            hnat = [persist.tile([P, D], BF16, name=f"hnat{j}",
                                 tag=f"hnat{j}") for j in range(nj)]
            onesrow = persist.tile([1, N], F32)
            nc.vector.memset(onesrow[:], 1.0)
            onescol = persist.tile([P, 1], BF16)
            nc.vector.memset(onescol[:], 1.0)
            f1row = persist.tile([1, NS], F32)
            frowA = persist.tile([2, N], F32)   # [f2 ; ones]
            frowB = persist.tile([2, NS], F32)  # [ones ; f1]
            nc.sync.dma_start(frowA[1:2, :], onesrow[:])
            nc.sync.dma_start(frowB[0:1, :], onesrow[:, 0:NS])

            # raw DMA'd weights + DVE-laundered copies (so matmuls never
            # depend on two DMA queues)
            WT_d = [persist.tile([D, D], F32, name=f"WTd{l}", tag=f"WTd{l}")
                    for l in range(nlayers)]
            bv_d = [persist.tile([D, 1], F32, name=f"bvd{l}", tag=f"bvd{l}")
                    for l in range(nlayers)]
            av_d = [persist.tile([D, 2], F32, name=f"avd{l}", tag=f"avd{l}")
                    for l in range(nlayers)]
            WtT_d = persist.tile([D, H], F32)
            btp_d = persist.tile([P, nH], F32)
            WT = [persist.tile([D, D], F32, name=f"WTl{l}", tag=f"WTl{l}")
                  for l in range(nlayers)]
            bv = [persist.tile([D, 1], F32, name=f"bvl{l}", tag=f"bvl{l}")
                  for l in range(nlayers)]
            av = [persist.tile([D, 2], F32, name=f"avl{l}", tag=f"avl{l}")
                  for l in range(nlayers)]
            WtTt = persist.tile([D, H], F32)
            btpt = persist.tile([P, nH], F32)
            for l in range(nlayers):
                nc.sync.dma_start(WT_d[l][:], WT_in[l].ap())
                nc.sync.dma_start(bv_d[l][:], bv_in[l].ap())
                nc.sync.dma_start(av_d[l][:], av_in[l].ap())
                nc.vector.tensor_copy(WT[l][:], WT_d[l][:])
                nc.vector.tensor_copy(bv[l][:], bv_d[l][:])
                nc.vector.tensor_copy(av[l][:], av_d[l][:])
            nc.sync.dma_start(WtT_d[:], WtT_in.ap())
            nc.sync.dma_start(btp_d[:], btp_in.ap())
            nc.vector.tensor_copy(WtTt[:], WtT_d[:])
            nc.vector.tensor_copy(btpt[:], btp_d[:])

            # ---- preprocessing: maskT[j][:, i] = 9e15*adj[i, j] - 9e15 ----
            # raw slots are "closed" with a full-coverage gpsimd memset so
            # the next DMA into the slot has exactly one wait.
            for itg in range(nit // 4):
                convs = []
                for q in range(4):
                    it = itg * 4 + q
                    raw = rawp.tile([P, N], I32, name=f"raw{it}", tag="raw")
                    nc.sync.dma_start(raw[:],
                                      adj_in.ap()[it * P:(it + 1) * P, :])
                    conv = convp.tile([P, N], BF16, name=f"conv{it}",
                                      tag="conv")
                    nc.vector.tensor_scalar(conv[:], raw[:], NEGC, -NEGC,
                                            OP.mult, OP.add)
                    nc.gpsimd.memset(raw[:], 0)
                    convs.append(conv)
                for j in range(nj):
                    # borrow the (idle) attention accumulator bank as a 3rd
                    # slot so the transpose->copy pipeline never stalls PE
                    if j % 3 == 2:
                        pst = attp.tile([P, 512], BF16,
                                        name=f"tp{itg}_{j}", tag="att")
                    else:
                        pst = pe_pool.tile([P, 512], BF16,
                                           name=f"tp{itg}_{j}", tag="pe")
                    for q in range(4):
                        nc.tensor.transpose(pst[:, q * P:(q + 1) * P],
                                            convs[q][:, j * P:(j + 1) * P],
                                            identb[:])
                    nc.vector.tensor_copy(
                        maskT[j][:, itg * 512:(itg + 1) * 512], pst[:])

            # ---- initial x state ----
            xT = xtp.tile([P, N], F32, name="xT0", tag="xT", bufs=3)
            nc.sync.dma_start(xT[:], xT_in.ap())
            xTs = xtp.tile([P, NS], F32, name="xTs0", tag="xTs")
            nc.sync.dma_start(xTs[:], xTs_in.ap())
            xs = []
            for k in range(nit):
                t = xsp.tile([P, D], F32, name=f"xs0_{k}", tag=f"xs{k}")
                nc.sync.dma_start(t[:], xs_in.ap()[k * P:(k + 1) * P, :])
                xs.append(t)

            for l in range(nlayers):
                last = l == nlayers - 1
                # h full (transposed): hT = relu(WT.T @ xT + b)
                for ch in range(nchN):
                    ps = btile([P, 512], F32, f"hps{l}_{ch}")
                    nc.tensor.matmul(ps[:], WT[l][:],
                                     xT[:, ch * 512:(ch + 1) * 512],
                                     start=True, stop=True)
                    nc.vector.tensor_scalar(hT[:, ch * 512:(ch + 1) * 512],
                                            ps[:], bv[l][:], 0.0,
                                            OP.add, OP.max)
                # h shard (transposed)
                for ch in range(nch):
                    ps = btile([P, 512], F32, f"hsps{l}_{ch}")
                    nc.tensor.matmul(ps[:], WT[l][:],
                                     xTs[:, ch * 512:(ch + 1) * 512],
                                     start=True, stop=True)
                    nc.vector.tensor_scalar(hsT[:, ch * 512:(ch + 1) * 512],
                                            ps[:], bv[l][:], 0.0,
                                            OP.add, OP.max)
                # f2 over all atoms / f1 over shard -> partition-0 rows
                for ch in range(nchN):
                    ps = btile([1, 512], F32, f"f2ps{l}_{ch}")
                    nc.tensor.matmul(ps[:], av[l][:, 1:2],
                                     hT[:, ch * 512:(ch + 1) * 512],
                                     start=True, stop=True)
                    nc.vector.tensor_copy(
                        frowA[0:1, ch * 512:(ch + 1) * 512], ps[0:1, :])
                for ch in range(nch):
                    ps = btile([1, 512], F32, f"f1ps{l}_{ch}")
                    nc.tensor.matmul(ps[:], av[l][:, 0:1],
                                     hsT[:, ch * 512:(ch + 1) * 512],
                                     start=True, stop=True)
                    nc.vector.tensor_copy(
                        f1row[0:1, ch * 512:(ch + 1) * 512], ps[0:1, :])
                nc.sync.dma_start(frowB[1:2, :], f1row[:])

                # hext: natural-layout h tiles (transpose hT) + ones column
                for g in range(nj // 4):
                    pst = btile([P, 512], F32, f"htp{l}_{g}")
                    for q in range(4):
                        j = g * 4 + q
                        nc.tensor.transpose(pst[:, q * P:(q + 1) * P],
                                            hT[:, j * P:(j + 1) * P],
                                            ident[:])
                    for q in range(4):
                        j = g * 4 + q
                        nc.vector.tensor_copy(hnat[j][:],
                                              pst[:, q * P:(q + 1) * P])

                # ---- attention + aggregation (transposed accum) ----
                psAT = attp.tile([P, NS], F32, name=f"psAT{l}", tag="att")
                psS = spp.tile([1, NS], F32, name=f"psS{l}", tag="s")
                for j in range(nj):
                    pe = pe_pool.tile([P, NS], F32, name=f"pe{l}_{j}",
                                      tag="pe")
                    for ch in range(nch):
                        sl = slice(ch * 512, (ch + 1) * 512)
                        # mask preload via PE identity-matmul (bf16)
                        nc.tensor.matmul(pe[:, sl], identb[:],
                                         maskT[j][:, sl],
                                         start=True, stop=False)
                        # += f2_j x ones + ones x f1_i  (K=2)
                        nc.tensor.matmul(pe[:, sl],
                                         frowA[0:2, j * P:(j + 1) * P],
                                         frowB[0:2, sl],
                                         start=False, stop=True)
                    # exp(leaky(z)) = max(exp(z), exp(0.01 z)), in bf16
                    q1 = qp.tile([P, NS], BF16, name=f"q1_{l}_{j}", tag="q1")
                    nc.scalar.activation(q1[:], pe[:], AF.Exp)
                    q2 = qp.tile([P, NS], BF16, name=f"q2_{l}_{j}", tag="q2")
                    nc.scalar.activation(q2[:], pe[:], AF.Exp, scale=0.01)
                    p = q1
                    nc.vector.tensor_tensor(p[:], q1[:], q2[:], OP.max)
                    for ch in range(nch):
                        sl = slice(ch * 512, (ch + 1) * 512)
                        nc.tensor.matmul(psAT[:, sl], hnat[j][:], p[:, sl],
                                         start=(j == 0), stop=(j == nj - 1))
                        nc.tensor.matmul(psS[0:1, sl], onescol[:], p[:, sl],
                                         start=(j == 0), stop=(j == nj - 1))

                # normalize + residual -> new xs tiles
                aT = qp.tile([P, NS], F32, name=f"aT{l}", tag="aT")
                nc.vector.tensor_copy(aT[:], psAT[:])
                s_row = smallp.tile([1, NS], F32, name=f"srow{l}",
                                    tag="srow")
                nc.vector.tensor_copy(s_row[:], psS[:])
                # s row -> per-partition column via PE transpose
                stp = pe_pool.tile([P, nit], F32, name=f"stp{l}", tag="pe")
                for it in range(nit):
                    nc.tensor.transpose(stp[:, it:it + 1],
                                        s_row[0:1, it * P:(it + 1) * P],
                                        ident[0:1, 0:1])
                rss = []
                for it in range(nit):
                    rs = smallp.tile([P, 1], F32, name=f"rs{l}_{it}",
                                     tag="rs", bufs=8)
                    nc.vector.reciprocal(rs[:], stp[:, it:it + 1])
                    rss.append(rs)
                xs_new = []
                for g2 in range(nit // 4):
                    atp = btile([P, 512], F32, f"atp{l}_{g2}")
                    for q in range(4):
                        it = g2 * 4 + q
                        nc.tensor.transpose(atp[:, q * P:(q + 1) * P],
                                            aT[:, it * P:(it + 1) * P],
                                            ident[:])
                    for q in range(4):
                        it = g2 * 4 + q
                        tmp = smallp.tile([P, D], F32, name=f"tmp{l}_{it}",
                                          tag="tmp")
                        nc.vector.tensor_scalar(
                            tmp[:], atp[:, q * P:(q + 1) * P],
                            rss[it][:], None, OP.mult)
                        xn = xsp.tile([P, D], F32, name=f"xs{l + 1}_{it}",
                                      tag=f"xs{it}")
                        nc.vector.tensor_tensor(xn[:], tmp[:], xs[it][:],
                                                OP.add)
                        xs_new.append(xn)
                xs = xs_new

                # transpose new shard -> xTs
                xTs = xtp.tile([P, NS], F32, name=f"xTs{l + 1}", tag="xTs")
                for g in range(nit // 4):
                    pst = btile([P, 512], F32, f"xtp{l}_{g}")
                    for q in range(4):
                        nc.tensor.transpose(pst[:, q * P:(q + 1) * P],
                                            xs[g * 4 + q][:], ident[:])
                    nc.vector.tensor_copy(xTs[:, g * 512:(g + 1) * 512],
                                          pst[:])

                if not last:
                    # exchange shards within the pair -> full xT
                    nc.gpsimd.dma_start(ag_in[l].ap(), xTs[:])
                    nc.gpsimd.collective_compute(
                        "AllGather", OP.bypass, replica_groups=pair_groups,
                        ins=[ag_in[l].ap()], outs=[ag_out[l].ap()])
                    xT = xtp.tile([P, N], F32, name=f"xT{l + 1}", tag="xT",
                                  bufs=3)
                    nc.gpsimd.dma_start(xT[:, 0:NS], ag_out[l].ap()[0:P, :])
                    nc.gpsimd.dma_start(xT[:, NS:N],
                                        ag_out[l].ap()[P:2 * P, :])

            # ---- final linear: out = relu(x @ Wt.T + bt) ----
            for g in range(nH):
                for ch in range(nch):
                    ps = btile([P, 512], F32, f"ops{g}_{ch}")
                    nc.tensor.matmul(ps[:], WtTt[:, g * P:(g + 1) * P],
                                     xTs[:, ch * 512:(ch + 1) * 512],
                                     start=True, stop=True)
                    oc = ocp.tile([P, 512], F32, name=f"oc{g}_{ch}",
                                  tag="oc")
                    nc.vector.tensor_scalar(oc[:], ps[:], btpt[:, g:g + 1],
                                            0.0, OP.add, OP.max)
                    pst = btile([P, 512], F32, f"otp{g}_{ch}")
                    for q in range(4):
                        nc.tensor.transpose(pst[:, q * P:(q + 1) * P],
                                            oc[:, q * P:(q + 1) * P],
                                            ident[:])
                    for q in range(4):
                        k = ch * 4 + q
                        ob = smallp.tile([P, P], F32, name=f"ob{g}_{k}",
                                         tag="ob")
                        nc.vector.tensor_copy(ob[:],
                                              pst[:, q * P:(q + 1) * P])
                        nc.sync.dma_start(
                            out_ext.ap()[k * P:(k + 1) * P,
                                         g * P:(g + 1) * P],
                            ob[:])

    if legalize:
        # semantics-preserving; skip for CoreSim runs (its race detector
        # rejects post-Tile instruction insertion)
        _legalize_waits(nc)
    return nc


def make_in_maps(x, adj, Ws, bs, avs, Wt, bt, num_cores, NS):
    """Per-core input dicts. Core c -> (graph c//2, row-half c%2)."""
    B, N, D = x.shape
    H = Wt.shape[0]
    nH = H // P
    x = np.ascontiguousarray(x, np.float32)
    adj = np.ascontiguousarray(adj, np.int32)
    shared = {"WtT": np.ascontiguousarray(np.asarray(Wt, np.float32).T),
              "btp": np.ascontiguousarray(
                  np.asarray(bt, np.float32).reshape(nH, P).T)}
    for l, (W, b, a) in enumerate(zip(Ws, bs, avs)):
        shared[f"WT{l}"] = np.ascontiguousarray(np.asarray(W, np.float32).T)
        shared[f"bv{l}"] = np.ascontiguousarray(
            np.asarray(b, np.float32).reshape(D, 1))
        shared[f"av{l}"] = np.ascontiguousarray(
            np.stack([np.asarray(a, np.float32)[:D, 0],
                      np.asarray(a, np.float32)[D:, 0]], axis=1))
    in_maps = []
    for c in range(num_cores):
        b, s = c // 2, c % 2
        m = dict(shared)
        m["xT"] = np.ascontiguousarray(x[b].T)
        m["xTs"] = np.ascontiguousarray(x[b, s * NS:(s + 1) * NS].T)
        m["xs"] = np.ascontiguousarray(x[b, s * NS:(s + 1) * NS])
        m["adj_s"] = np.ascontiguousarray(adj[b, s * NS:(s + 1) * NS, :])
        in_maps.append(m)
    return in_maps


_NC_CACHE = {}


def kernel(x, adj, W0, b0, W1, b1, W2, b2, a0, a1, a2, Wt, bt):
    B, N, D = 4, 2048, 128
    H = 256
    NUM_CORES = 8
    NS = N // 2
    pair_groups = [[2 * i, 2 * i + 1] for i in range(NUM_CORES // 2)]

    key = (N, NS, D, H, NUM_CORES)
    if key not in _NC_CACHE:
        _NC_CACHE[key] = build_gat_nc(N, NS, D, H, NUM_CORES, pair_groups)
    nc = _NC_CACHE[key]

    in_maps = make_in_maps(np.asarray(x), np.asarray(adj),
                           [W0, W1, W2], [b0, b1, b2], [a0, a1, a2],
                           np.asarray(Wt), np.asarray(bt), NUM_CORES, NS)
    res = run_bass_kernel_spmd(nc, in_maps, list(range(NUM_CORES))).results
    out = np.empty((B, N, H), np.float32)
    for c in range(NUM_CORES):
        b, s = c // 2, c % 2
        out[b, s * NS:(s + 1) * NS, :] = res[c]["out_s"]
    return out



# revision 5
# speedup vs baseline: 2.2415x; 2.2415x over previous
"""GAT (3-layer graph attention + final linear) Trainium2 Bass kernel.

Problem: B=4 graphs, N=2048 atoms, D=128, H=256.
  per layer: h = relu(x @ W.T + b); e_ij = leaky_relu(f1_i + f2_j, 0.01)
  masked by adj; att = softmax_j(e); x = x + att @ h.
  final: relu(x @ Wt.T + bt).

Sharding: 8 cores; core c -> (graph b=c//2, row-half s=c%2 of the NxN
attention). Each core computes attention only for its 1024 rows (i),
over all 2048 columns (j). Per layer, each core computes h for its own
rows and the full h is assembled with a pair AllGather (bf16 payload).

Key structure (all heavy work in bf16 on the PE, logits on ACT):
  - adj is transposed on the HOST and fed as a bf16 0/1 mask in
    [j, i-shard] layout -- no on-device transposes or int conversion.
  - logits are built INSIDE the activation pass: t = Prelu(f1bc + f2col_j)
    with f1 broadcast once per layer (gpsimd partition_broadcast) and
    f2_j as the per-partition bias; then q = Exp(t). exp(leaky(z)) needs
    no row-max: logits are bounded (~36) so f32 exp is safe.
  - mask applied multiplicatively after exp on DVE (bf16, 2x mode).
  - aggregation transposed: psAT[d,i] += hnat_j^T @ p_j; row sums via a
    ones-column matmul into psS[1,i]; normalize + residual done fully in
    the transposed layout (recip row -> partition_broadcast -> two DVE
    tensor_tensor ops), so x never leaves [d, i] layout.
  - output written transposed [H, NS]; host transposes back.
"""

import numpy as np
import ml_dtypes

import concourse.bass as bass
import concourse.mybir as mybir
import concourse.tile as tile
from concourse import masks
from concourse.bass_utils import run_bass_kernel_spmd

P = 128
F32 = mybir.dt.float32
BF16 = mybir.dt.bfloat16
AF = mybir.ActivationFunctionType
OP = mybir.AluOpType


def _legalize_waits(nc, dma_limit=1, engine_limit=1):
    """Walrus can encode only 1 sem wait on a DMA instruction and ~2 on an
    engine instruction. Move excess waits onto standalone EventSemaphore
    instructions (1 wait each) inserted just before the offender on the
    same engine."""
    counter = [0]

    def split(ins):
        si = ins.sync_info
        if si is None:
            return None
        limit = dma_limit if type(ins).__name__.startswith("InstDMA") \
            else engine_limit
        waits = list(si.on_wait)
        if len(waits) <= limit:
            return None
        keep = waits[-limit:] if limit > 0 else []
        extra = waits[:-limit] if limit > 0 else waits
        evs = []
        for w in extra:
            counter[0] += 1
            evs.append(mybir.InstEventSemaphore(
                name=f"evsplit{counter[0]}", engine=ins.engine,
                sync_info=mybir.SyncInfo(on_wait=[w], on_update=[])))
        ins.sync_info = mybir.SyncInfo(on_wait=keep,
                                       on_update=list(si.on_update))
        return evs

    for f in nc.m.functions:
        for blk in f.blocks:
            new_list = []
            changed = False
            for ins in blk.instructions:
                evs = split(ins)
                if evs:
                    new_list.extend(evs)
                    changed = True
                new_list.append(ins)
            if changed:
                blk.instructions = new_list


def build_gat_nc(N, NS, D, H, num_cores, pair_groups, nlayers=3,
                 legalize=True):
    assert D == P and NS % 512 == 0 and N % 512 == 0
    nj = N // P        # j tiles (16)
    nch = NS // 512    # 512-chunks in the i shard (2)
    nH = H // P

    nc = bass.Bass("TRN2", target_bir_lowering=False, debug=False,
                   num_devices=num_cores)

    # ---- I/O ----
    xTs_in = nc.dram_tensor("xTs", [P, NS], F32, kind="ExternalInput")
    xTsb_in = nc.dram_tensor("xTsb", [P, NS], BF16, kind="ExternalInput")
    xTb_in = nc.dram_tensor("xTb", [P, N], BF16, kind="ExternalInput")
    mask_in = nc.dram_tensor("maskTb", [N, NS], BF16, kind="ExternalInput")
    WT_in = [nc.dram_tensor(f"WT{l}", [D, D], BF16, kind="ExternalInput")
             for l in range(nlayers)]
    bv_in = [nc.dram_tensor(f"bv{l}", [D, 1], F32, kind="ExternalInput")
             for l in range(nlayers)]
    av_in = [nc.dram_tensor(f"av{l}", [D, 2], BF16, kind="ExternalInput")
             for l in range(nlayers)]
    WtT_in = nc.dram_tensor("WtT", [D, H], BF16, kind="ExternalInput")
    btp_in = nc.dram_tensor("btp", [P, nH], F32, kind="ExternalInput")
    out_ext = nc.dram_tensor("outT_s", [H, NS], F32, kind="ExternalOutput")

    # DRAM bounce buffers for the pair AllGather of h shards (layers 1..)
    ag_in = [None] + [nc.dram_tensor(f"ag_in{l}", [P, NS], BF16)
                      for l in range(1, nlayers)]
    ag_out = [None] + [nc.dram_tensor(f"ag_out{l}", [2 * P, NS], BF16)
                       for l in range(1, nlayers)]

    with tile.TileContext(nc) as tc:
        import contextlib
        ctx = contextlib.ExitStack()
        with ctx:
            persist = ctx.enter_context(tc.tile_pool(name="persist", bufs=1))
            htp = ctx.enter_context(tc.tile_pool(name="htp", bufs=2))
            hsp = ctx.enter_context(tc.tile_pool(name="hsp", bufs=2))
            xtp = ctx.enter_context(tc.tile_pool(name="xtp", bufs=2))
            fbp = ctx.enter_context(tc.tile_pool(name="fbp", bufs=2))
            qp = ctx.enter_context(tc.tile_pool(name="qp", bufs=2))
            hnp = ctx.enter_context(tc.tile_pool(name="hnp", bufs=2))
            smallp = ctx.enter_context(tc.tile_pool(name="smallp", bufs=2))
            ocp = ctx.enter_context(tc.tile_pool(name="ocp", bufs=2))
            pe_pool = ctx.enter_context(
                tc.tile_pool(name="pe_pool", bufs=4, space="PSUM"))
            attp = ctx.enter_context(
                tc.tile_pool(name="attp", bufs=1, space="PSUM"))
            spp = ctx.enter_context(
                tc.tile_pool(name="spp", bufs=1, space="PSUM"))

            identb = persist.tile([P, P], BF16)
            masks.make_identity(nc, identb[:])
            onescol = persist.tile([P, 1], BF16)
            nc.vector.memset(onescol[:], 1.0)
            onesrow = persist.tile([1, P], F32)
            nc.vector.memset(onesrow[:], 1.0)
            F32R = mybir.dt.float32r

            def bcast_row(row, dst, tag):
                """dst[p, i] = row[0, i] via K=1 ones matmul (fp32)."""
                n = dst.shape[1]
                for ch in range(n // 512):
                    sl = slice(ch * 512, (ch + 1) * 512)
                    ps = pe_pool.tile([P, 512], F32, name=f"bc_{tag}_{ch}",
                                      tag="pe")
                    nc.tensor.matmul(ps[:], onesrow[:], row[0:1, sl],
                                     start=True, stop=True)
                    nc.vector.tensor_copy(dst[:, sl], ps[:])

            # ---- weights (one DMA each, read-only) ----
            WT = [persist.tile([D, D], BF16, name=f"WT{l}", tag=f"WT{l}")
                  for l in range(nlayers)]
            bv = [persist.tile([D, 1], F32, name=f"bv{l}", tag=f"bv{l}")
                  for l in range(nlayers)]
            av = [persist.tile([D, 2], BF16, name=f"av{l}", tag=f"av{l}")
                  for l in range(nlayers)]
            WtTt = persist.tile([D, H], BF16)
            btpt = persist.tile([P, nH], F32)
            for l in range(nlayers):
                nc.sync.dma_start(WT[l][:], WT_in[l].ap())
                nc.sync.dma_start(bv[l][:], bv_in[l].ap())
                nc.sync.dma_start(av[l][:], av_in[l].ap())
            nc.sync.dma_start(WtTt[:], WtT_in.ap())
            nc.sync.dma_start(btpt[:], btp_in.ap())

            # ---- initial x state (transposed f32 + bf16) ----
            xTs = xtp.tile([P, NS], F32, name="xTs0", tag="xTs")
            nc.sync.dma_start(xTs[:], xTs_in.ap())
            xTsb = xtp.tile([P, NS], BF16, name="xTsb0", tag="xTsb")
            nc.sync.dma_start(xTsb[:], xTsb_in.ap())
            xTb = persist.tile([P, N], BF16)
            nc.sync.dma_start(xTb[:], xTb_in.ap())

            # ---- adjacency mask tiles (bf16 0/1, [j, i] layout) ----
            maskM = [persist.tile([P, NS], BF16, name=f"maskM{j}",
                                  tag=f"maskM{j}") for j in range(nj)]
            for j in range(nj):
                nc.sync.dma_start(maskM[j][:],
                                  mask_in.ap()[j * P:(j + 1) * P, :])

            for l in range(nlayers):
                # -- local h (rows of this shard), bf16 [P, NS] --
                hsT = hsp.tile([P, NS], BF16, name=f"hsT{l}", tag="hsT")
                for ch in range(nch):
                    sl = slice(ch * 512, (ch + 1) * 512)
                    ps = pe_pool.tile([P, 512], F32, name=f"hps{l}_{ch}",
                                      tag="pe")
                    nc.tensor.matmul(ps[:], WT[l][:], xTsb[:, sl],
                                     start=True, stop=True)
                    nc.vector.tensor_scalar(hsT[:, sl], ps[:], bv[l][:],
                                            0.0, OP.add, OP.max)
                # -- f1 over the shard rows (from local h) --
                f1row = smallp.tile([1, NS], F32, name=f"f1row{l}",
                                    tag="f1row")
                for ch in range(nch):
                    sl = slice(ch * 512, (ch + 1) * 512)
                    psf = pe_pool.tile([2, 512], F32, name=f"fps{l}_{ch}",
                                       tag="pe")
                    nc.tensor.matmul(psf[:], av[l][:], hsT[:, sl],
                                     start=True, stop=True)
                    nc.vector.tensor_copy(f1row[0:1, sl], psf[0:1, :])
                f1bc = fbp.tile([P, NS], F32, name=f"f1bc{l}", tag="f1bc")
                bcast_row(f1row, f1bc, f"f1_{l}")

                # -- full h (transposed bf16 [P, N]) --
                hT = htp.tile([P, N], BF16, name=f"hT{l}", tag="hT")
                if l == 0:
                    for ch in range(N // 512):
                        sl = slice(ch * 512, (ch + 1) * 512)
                        ps = pe_pool.tile([P, 512], F32,
                                          name=f"hfps{l}_{ch}", tag="pe")
                        nc.tensor.matmul(ps[:], WT[l][:], xTb[:, sl],
                                         start=True, stop=True)
                        nc.vector.tensor_scalar(hT[:, sl], ps[:], bv[l][:],
                                                0.0, OP.add, OP.max)
                else:
                    nc.gpsimd.dma_start(ag_in[l].ap(), hsT[:])
                    nc.gpsimd.collective_compute(
                        "AllGather", OP.bypass, replica_groups=pair_groups,
                        ins=[ag_in[l].ap()], outs=[ag_out[l].ap()])
                    nc.gpsimd.dma_start(hT[:, 0:NS], ag_out[l].ap()[0:P, :])
                    nc.gpsimd.dma_start(hT[:, NS:N],
                                        ag_out[l].ap()[P:2 * P, :])

                # -- f2 columns per j tile: [f1col_t, f2col_t] via
                #    hT_tile^T @ av  (tiny moving, 2 cols) --
                f2c = []
                for g in range(nj // 4):
                    psc = pe_pool.tile([P, 8], F32, name=f"psc{l}_{g}",
                                       tag="pe")
                    for q in range(4):
                        t = g * 4 + q
                        nc.tensor.matmul(psc[:, 2 * q:2 * q + 2],
                                         hT[:, t * P:(t + 1) * P], av[l][:],
                                         start=True, stop=True)
                    fc = smallp.tile([P, 8], F32, name=f"f2c{l}_{g}",
                                     tag=f"f2c{g}")
                    nc.vector.tensor_copy(fc[:], psc[:])
                    f2c.append(fc)

                # -- hnat: natural-layout h tiles (transpose hT, bf16) --
                hnatg = []
                for g in range(nj // 4):
                    pst = pe_pool.tile([P, 512], BF16, name=f"htp{l}_{g}",
                                       tag="pe")
                    for q in range(4):
                        t = g * 4 + q
                        nc.tensor.transpose(pst[:, q * P:(q + 1) * P],
                                            hT[:, t * P:(t + 1) * P],
                                            identb[:])
                    hn = hnp.tile([P, 512], BF16, name=f"hng{l}_{g}",
                                  tag=f"hng{g}")
                    nc.vector.tensor_copy(hn[:], pst[:])
                    hnatg.append(hn)

                # ---- attention: logits on ACT, mask on DVE, agg on PE ----
                psAT = attp.tile([P, NS], F32, name=f"psAT{l}", tag="att")
                psS = spp.tile([1, NS], F32, name=f"psS{l}", tag="s")
                for t in range(nj):
                    g, q = t // 4, t % 4
                    tf = qp.tile([P, NS], F32, name=f"tf{l}_{t}", tag="tf")
                    nc.scalar.activation(tf[:], f1bc[:], AF.Prelu,
                                         bias=f2c[g][:, 2 * q + 1:2 * q + 2],
                                         scale=1.0, alpha=0.01)
                    qb = qp.tile([P, NS], BF16, name=f"qb{l}_{t}", tag="qb")
                    nc.scalar.activation(qb[:], tf[:], AF.Exp)
                    pb = qp.tile([P, NS], BF16, name=f"pb{l}_{t}", tag="pb")
                    nc.vector.tensor_tensor(pb[:], qb[:], maskM[t][:],
                                            OP.mult)
                    for ch in range(nch):
                        sl = slice(ch * 512, (ch + 1) * 512)
                        nc.tensor.matmul(psAT[:, sl],
                                         hnatg[g][:, q * P:(q + 1) * P],
                                         pb[:, sl],
                                         start=(t == 0), stop=(t == nj - 1))
                        nc.tensor.matmul(psS[0:1, sl], onescol[:], pb[:, sl],
                                         start=(t == 0), stop=(t == nj - 1))

                # ---- normalize + residual, fully transposed ----
                rrow = smallp.tile([1, NS], F32, name=f"rrow{l}", tag="rrow")
                nc.vector.reciprocal(rrow[:], psS[:])
                rbc = fbp.tile([P, NS], F32, name=f"rbc{l}", tag="rbc")
                bcast_row(rrow, rbc, f"r_{l}")
                tmp = qp.tile([P, NS], F32, name=f"tmp{l}", tag="tf")
                nc.vector.tensor_tensor(tmp[:], psAT[:], rbc[:], OP.mult)
                xTs_new = xtp.tile([P, NS], F32, name=f"xTs{l + 1}",
                                   tag="xTs")
                nc.vector.tensor_tensor(xTs_new[:], tmp[:], xTs[:], OP.add)
                xTs = xTs_new
                xTsb = xtp.tile([P, NS], BF16, name=f"xTsb{l + 1}",
                                tag="xTsb")
                nc.vector.tensor_copy(xTsb[:], xTs[:])

            # ---- final linear: outT = relu(WtT^T @ xTsb + bt) ----
            for g in range(nH):
                for ch in range(nch):
                    sl = slice(ch * 512, (ch + 1) * 512)
                    ps = pe_pool.tile([P, 512], F32, name=f"ops{g}_{ch}",
                                      tag="pe")
                    nc.tensor.matmul(ps[:], WtTt[:, g * P:(g + 1) * P],
                                     xTsb[:, sl], start=True, stop=True)
                    oc = ocp.tile([P, 512], F32, name=f"oc{g}_{ch}",
                                  tag="oc")
                    nc.vector.tensor_scalar(oc[:], ps[:], btpt[:, g:g + 1],
                                            0.0, OP.add, OP.max)
                    nc.sync.dma_start(
                        out_ext.ap()[g * P:(g + 1) * P, sl], oc[:])

    if legalize:
        _legalize_waits(nc)
    return nc


def make_in_maps(x, adj, Ws, bs, avs, Wt, bt, num_cores, NS):
    """Per-core input dicts. Core c -> (graph c//2, row-half c%2)."""
    B, N, D = x.shape
    H = Wt.shape[0]
    nH = H // P
    x = np.ascontiguousarray(x, np.float32)
    adj = np.asarray(adj)
    shared = {"WtT": np.ascontiguousarray(
                  np.asarray(Wt, np.float32).T).astype(ml_dtypes.bfloat16),
              "btp": np.ascontiguousarray(
                  np.asarray(bt, np.float32).reshape(nH, P).T)}
    for l, (W, b, a) in enumerate(zip(Ws, bs, avs)):
        shared[f"WT{l}"] = np.ascontiguousarray(
            np.asarray(W, np.float32).T).astype(ml_dtypes.bfloat16)
        shared[f"bv{l}"] = np.ascontiguousarray(
            np.asarray(b, np.float32).reshape(D, 1))
        shared[f"av{l}"] = np.ascontiguousarray(
            np.stack([np.asarray(a, np.float32)[:D, 0],
                      np.asarray(a, np.float32)[D:, 0]],
                     axis=1)).astype(ml_dtypes.bfloat16)
    in_maps = []
    for c in range(num_cores):
        b, s = c // 2, c % 2
        m = dict(shared)
        xT = np.ascontiguousarray(x[b].T)
        m["xTs"] = np.ascontiguousarray(xT[:, s * NS:(s + 1) * NS])
        m["xTsb"] = m["xTs"].astype(ml_dtypes.bfloat16)
        m["xTb"] = xT.astype(ml_dtypes.bfloat16)
        m["maskTb"] = np.ascontiguousarray(
            adj[b, s * NS:(s + 1) * NS, :].T).astype(ml_dtypes.bfloat16)
        in_maps.append(m)
    return in_maps


_NC_CACHE = {}


def kernel(x, adj, W0, b0, W1, b1, W2, b2, a0, a1, a2, Wt, bt):
    B, N, D = 4, 2048, 128
    H = 256
    NUM_CORES = 8
    NS = N // 2
    pair_groups = [[2 * i, 2 * i + 1] for i in range(NUM_CORES // 2)]

    key = (N, NS, D, H, NUM_CORES)
    if key not in _NC_CACHE:
        _NC_CACHE[key] = build_gat_nc(N, NS, D, H, NUM_CORES, pair_groups)
    nc = _NC_CACHE[key]

    in_maps = make_in_maps(np.asarray(x), np.asarray(adj),
                           [W0, W1, W2], [b0, b1, b2], [a0, a1, a2],
                           np.asarray(Wt), np.asarray(bt), NUM_CORES, NS)
    res = run_bass_kernel_spmd(nc, in_maps, list(range(NUM_CORES))).results
    out = np.empty((B, N, H), np.float32)
    for c in range(NUM_CORES):
        b, s = c // 2, c % 2
        out[b, s * NS:(s + 1) * NS, :] = res[c]["outT_s"].T
    return out


# revision 8
# speedup vs baseline: 2.3188x; 1.0345x over previous
"""GAT (3-layer graph attention + final linear) Trainium2 Bass kernel.

Problem: B=4 graphs, N=2048 atoms, D=128, H=256.
  per layer: h = relu(x @ W.T + b); e_ij = leaky_relu(f1_i + f2_j, 0.01)
  masked by adj; att = softmax_j(e); x = x + att @ h.
  final: relu(x @ Wt.T + bt).

Sharding: 8 cores; core c -> (graph b=c//2, row-half s=c%2 of the NxN
attention). Each core computes attention for its 1024 rows (i) over all
2048 columns (j), in a core-local [local|remote] column layout: j tiles
0-7 are the core's OWN rows (h computed locally), 8-15 the partner's
(arriving via a pair AllGather). The host permutes the mask and xT
inputs to match, so the program is SPMD-uniform and the attention loop
naturally runs local tiles while the collective is in flight.

Key structure (all matmuls bf16, logits fused into ACT):
  - adj transposed on the HOST into a bf16 0/1 mask, [j, i] layout,
    j-tiles permuted local-first per core.
  - logits built inside the activation pass: t = Prelu(f1bc + f2col_j,
    alpha=.01), q = Exp(t); both share one ACT table set. No row-max
    needed: logits are bounded (~36) so f32 exp is safe.
  - mask applied multiplicatively after exp on DVE (bf16).
  - aggregation transposed: psAT[d,i] += hnat_j^T @ p_j; row sums via a
    ones-column matmul into psS[1,i]. Normalize + residual stay in the
    transposed layout: 1/s computed in a [128,8] column shape (DVE
    reciprocal is ~6.4ns/elem, so never on [1,N] rows), broadcast back
    over partitions with tiny ones-matmuls.
  - AllGather import is SPMD-uniform: both halves of ag_out are DMA'd in
    and blended with a host-fed per-core 0/1 selector (3 DVE ops).
  - output written transposed [H, NS]; host transposes back.
"""

import numpy as np
import ml_dtypes

import concourse.bass as bass
import concourse.mybir as mybir
import concourse.tile as tile
from concourse import masks
from concourse.bass_utils import run_bass_kernel_spmd

P = 128
F32 = mybir.dt.float32
BF16 = mybir.dt.bfloat16
AF = mybir.ActivationFunctionType
OP = mybir.AluOpType


def _legalize_waits(nc, dma_limit=1, engine_limit=1):
    """Walrus can encode only 1 sem wait on a DMA instruction and ~2 on an
    engine instruction. Move excess waits onto standalone EventSemaphore
    instructions (1 wait each) inserted just before the offender on the
    same engine."""
    counter = [0]

    def split(ins):
        si = ins.sync_info
        if si is None:
            return None
        limit = dma_limit if type(ins).__name__.startswith("InstDMA") \
            else engine_limit
        waits = list(si.on_wait)
        if len(waits) <= limit:
            return None
        keep = waits[-limit:] if limit > 0 else []
        extra = waits[:-limit] if limit > 0 else waits
        evs = []
        for w in extra:
            counter[0] += 1
            evs.append(mybir.InstEventSemaphore(
                name=f"evsplit{counter[0]}", engine=ins.engine,
                sync_info=mybir.SyncInfo(on_wait=[w], on_update=[])))
        ins.sync_info = mybir.SyncInfo(on_wait=keep,
                                       on_update=list(si.on_update))
        return evs

    for f in nc.m.functions:
        for blk in f.blocks:
            new_list = []
            changed = False
            for ins in blk.instructions:
                evs = split(ins)
                if evs:
                    new_list.extend(evs)
                    changed = True
                new_list.append(ins)
            if changed:
                blk.instructions = new_list


def build_gat_nc(N, NS, D, H, num_cores, pair_groups, nlayers=3,
                 legalize=True):
    assert D == P and NS % 512 == 0 and N % 512 == 0
    nj = N // P        # j tiles (16)
    njl = nj // 2      # local j tiles (8)
    nch = NS // 512    # 512-chunks in the i shard (2)
    nit = NS // P      # i tiles (8)
    nH = H // P

    nc = bass.Bass("TRN2", target_bir_lowering=False, debug=False,
                   num_devices=num_cores)

    # ---- I/O ----
    xTsb_in = nc.dram_tensor("xTsb", [P, NS], BF16, kind="ExternalInput")
    xTs_in = nc.dram_tensor("xTs", [P, NS], F32, kind="ExternalInput")
    xTb_in = nc.dram_tensor("xTb", [P, N], BF16, kind="ExternalInput")
    mask_in = nc.dram_tensor("maskTb", [N, NS], BF16, kind="ExternalInput")
    sel_in = nc.dram_tensor("sel", [P, 2], F32, kind="ExternalInput")
    WT_in = [nc.dram_tensor(f"WT{l}", [D, D], BF16, kind="ExternalInput")
             for l in range(nlayers)]
    bv_in = [nc.dram_tensor(f"bv{l}", [D, 1], F32, kind="ExternalInput")
             for l in range(nlayers)]
    av_in = [nc.dram_tensor(f"av{l}", [D, 2], BF16, kind="ExternalInput")
             for l in range(nlayers)]
    WtT_in = nc.dram_tensor("WtT", [D, H], BF16, kind="ExternalInput")
    btp_in = nc.dram_tensor("btp", [P, nH], F32, kind="ExternalInput")
    out_ext = nc.dram_tensor("outT_s", [H, NS], F32, kind="ExternalOutput")

    # DRAM bounce buffers for the pair AllGather of h shards (layers 1..)
    ag_in = [None] + [nc.dram_tensor(f"ag_in{l}", [P, NS], BF16)
                      for l in range(1, nlayers)]
    ag_out = [None] + [nc.dram_tensor(f"ag_out{l}", [2 * P, NS], BF16)
                       for l in range(1, nlayers)]

    with tile.TileContext(nc) as tc:
        import contextlib
        ctx = contextlib.ExitStack()
        with ctx:
            persist = ctx.enter_context(tc.tile_pool(name="persist", bufs=1))
            htp = ctx.enter_context(tc.tile_pool(name="htp", bufs=2))
            hgp = ctx.enter_context(tc.tile_pool(name="hgp", bufs=2))
            xtp = ctx.enter_context(tc.tile_pool(name="xtp", bufs=2))
            fbp = ctx.enter_context(tc.tile_pool(name="fbp", bufs=2))
            qp = ctx.enter_context(tc.tile_pool(name="qp", bufs=2))
            hnp = ctx.enter_context(tc.tile_pool(name="hnp", bufs=2))
            smallp = ctx.enter_context(tc.tile_pool(name="smallp", bufs=2))
            ocp = ctx.enter_context(tc.tile_pool(name="ocp", bufs=2))
            pe_pool = ctx.enter_context(
                tc.tile_pool(name="pe_pool", bufs=4, space="PSUM"))
            attp = ctx.enter_context(
                tc.tile_pool(name="attp", bufs=1, space="PSUM"))
            spp = ctx.enter_context(
                tc.tile_pool(name="spp", bufs=1, space="PSUM"))

            identb = persist.tile([P, P], BF16)
            masks.make_identity(nc, identb[:])
            identf = persist.tile([P, P], F32)
            masks.make_identity(nc, identf[:])
            onescol = persist.tile([P, 1], BF16)
            nc.vector.memset(onescol[:], 1.0)
            onesrow = persist.tile([1, P], F32)
            nc.vector.memset(onesrow[:], 1.0)
            onesrowb = persist.tile([1, P], BF16)
            nc.vector.memset(onesrowb[:], 1.0)

            # ---- weights / selector (one DMA each, read-only) ----
            WT = [persist.tile([D, D], BF16, name=f"WT{l}", tag=f"WT{l}")
                  for l in range(nlayers)]
            bv = [persist.tile([D, 1], F32, name=f"bv{l}", tag=f"bv{l}")
                  for l in range(nlayers)]
            av = [persist.tile([D, 2], BF16, name=f"av{l}", tag=f"av{l}")
                  for l in range(nlayers)]
            WtTt = persist.tile([D, H], BF16)
            btpt = persist.tile([P, nH], F32)
            selt = persist.tile([P, 2], F32)
            for l in range(nlayers):
                nc.sync.dma_start(WT[l][:], WT_in[l].ap())
                nc.sync.dma_start(bv[l][:], bv_in[l].ap())
                nc.sync.dma_start(av[l][:], av_in[l].ap())
            nc.sync.dma_start(WtTt[:], WtT_in.ap())
            nc.sync.dma_start(btpt[:], btp_in.ap())
            nc.sync.dma_start(selt[:], sel_in.ap())

            # ---- initial x state (transposed bf16 + f32 residual) ----
            xTsb = xtp.tile([P, NS], BF16, name="xTsb0", tag="xTsb")
            nc.sync.dma_start(xTsb[:], xTsb_in.ap())
            xTs = xtp.tile([P, NS], F32, name="xTs0", tag="xTs")
            nc.sync.dma_start(xTs[:], xTs_in.ap())
            xTb = persist.tile([P, N], BF16)
            nc.sync.dma_start(xTb[:], xTb_in.ap())

            # ---- adjacency mask tiles (bf16 0/1, [j, i], local-first) ----
            maskM = [persist.tile([P, NS], BF16, name=f"maskM{j}",
                                  tag=f"maskM{j}") for j in range(nj)]
            for j in range(nj):
                nc.sync.dma_start(maskM[j][:],
                                  mask_in.ap()[j * P:(j + 1) * P, :])

            for l in range(nlayers):
                last = l == nlayers - 1
                hT = htp.tile([P, N], BF16, name=f"hT{l}", tag="hT")
                # -- local h -> hT[:, 0:NS] --
                loc_src = xTb if l == 0 else xTsb
                for ch in range(nch):
                    sl = slice(ch * 512, (ch + 1) * 512)
                    ps = pe_pool.tile([P, 512], F32, name=f"hps{l}_{ch}",
                                      tag="pe")
                    nc.tensor.matmul(ps[:], WT[l][:], loc_src[:, sl],
                                     start=True, stop=True)
                    nc.vector.tensor_scalar(hT[:, sl], ps[:], bv[l][:],
                                            0.0, OP.add, OP.max)
                # -- launch remote-h exchange (or compute locally at l=0) --
                if l == 0:
                    for ch in range(nch):
                        sl = slice(NS + ch * 512, NS + (ch + 1) * 512)
                        ps = pe_pool.tile([P, 512], F32,
                                          name=f"hrps{l}_{ch}", tag="pe")
                        nc.tensor.matmul(ps[:], WT[l][:], xTb[:, sl],
                                         start=True, stop=True)
                        nc.vector.tensor_scalar(hT[:, sl], ps[:], bv[l][:],
                                                0.0, OP.add, OP.max)
                else:
                    nc.gpsimd.dma_start(ag_in[l].ap(), hT[:, 0:NS])
                    nc.gpsimd.collective_compute(
                        "AllGather", OP.bypass, replica_groups=pair_groups,
                        ins=[ag_in[l].ap()], outs=[ag_out[l].ap()])
                    hg0 = hgp.tile([P, NS], BF16, name=f"hg0_{l}", tag="hg0")
                    hg1 = hgp.tile([P, NS], BF16, name=f"hg1_{l}", tag="hg1")
                    nc.gpsimd.dma_start(hg0[:], ag_out[l].ap()[0:P, :])
                    nc.gpsimd.dma_start(hg1[:], ag_out[l].ap()[P:2 * P, :])

                # -- f1 over shard rows (from local h half) --
                f1row = smallp.tile([1, NS], F32, name=f"f1row{l}",
                                    tag="f1row")
                for ch in range(nch):
                    sl = slice(ch * 512, (ch + 1) * 512)
                    psf = pe_pool.tile([2, 512], F32, name=f"fps{l}_{ch}",
                                       tag="pe")
                    nc.tensor.matmul(psf[:], av[l][:], hT[:, sl],
                                     start=True, stop=True)
                    nc.vector.tensor_copy(f1row[0:1, sl], psf[0:1, :])
                f1bc = fbp.tile([P, NS], F32, name=f"f1bc{l}", tag="f1bc")
                for ch in range(nch):
                    sl = slice(ch * 512, (ch + 1) * 512)
                    ps = pe_pool.tile([P, 512], F32, name=f"bcf{l}_{ch}",
                                      tag="pe")
                    nc.tensor.matmul(ps[:], onesrow[:], f1row[0:1, sl],
                                     start=True, stop=True)
                    nc.vector.tensor_copy(f1bc[:, sl], ps[:])

                # -- per-j-tile [f1col, f2col] and natural-layout h --
                f2c = [None] * (nj // 4)
                hnatg = [None] * (nj // 4)

                def prep_group(g, l=l, hT=hT, f2c=f2c, hnatg=hnatg):
                    psc = pe_pool.tile([P, 8], F32, name=f"psc{l}_{g}",
                                       tag="pe")
                    for q in range(4):
                        t = g * 4 + q
                        nc.tensor.matmul(psc[:, 2 * q:2 * q + 2],
                                         hT[:, t * P:(t + 1) * P], av[l][:],
                                         start=True, stop=True)
                    fc = smallp.tile([P, 8], F32, name=f"f2c{l}_{g}",
                                     tag=f"f2c{g}")
                    nc.vector.tensor_copy(fc[:], psc[:])
                    f2c[g] = fc
                    pst = pe_pool.tile([P, 512], BF16, name=f"htp{l}_{g}",
                                       tag="pe")
                    for q in range(4):
                        t = g * 4 + q
                        nc.tensor.transpose(pst[:, q * P:(q + 1) * P],
                                            hT[:, t * P:(t + 1) * P],
                                            identb[:])
                    hn = hnp.tile([P, 512], BF16, name=f"hng{l}_{g}",
                                  tag=f"hng{g}")
                    nc.vector.tensor_copy(hn[:], pst[:])
                    hnatg[g] = hn

                prep_group(0)
                prep_group(1)
                if l == 0:
                    prep_group(2)
                    prep_group(3)

                # ---- attention: logits on ACT, mask on DVE, agg on PE ----
                psAT = attp.tile([P, NS], F32, name=f"psAT{l}", tag="att")
                psS = spp.tile([1, NS], F32, name=f"psS{l}", tag="s")

                def att_tile(t, l=l, psAT=psAT, psS=psS, f2c=f2c,
                             hnatg=hnatg, f1bc=f1bc):
                    g, q = t // 4, t % 4
                    tf = qp.tile([P, NS], F32, name=f"tf{l}_{t}", tag="tf")
                    nc.scalar.activation(tf[:], f1bc[:], AF.Prelu,
                                         bias=f2c[g][:, 2 * q + 1:2 * q + 2],
                                         scale=1.0, alpha=0.01)
                    qb = qp.tile([P, NS], BF16, name=f"qb{l}_{t}", tag="qb")
                    nc.scalar.activation(qb[:], tf[:], AF.Exp)
                    pb = qp.tile([P, NS], BF16, name=f"pb{l}_{t}", tag="pb")
                    nc.vector.tensor_tensor(pb[:], qb[:], maskM[t][:],
                                            OP.mult)
                    for ch in range(nch):
                        sl = slice(ch * 512, (ch + 1) * 512)
                        nc.tensor.matmul(psAT[:, sl],
                                         hnatg[g][:, q * P:(q + 1) * P],
                                         pb[:, sl],
                                         start=(t == 0), stop=(t == nj - 1))
                        nc.tensor.matmul(psS[0:1, sl], onescol[:], pb[:, sl],
                                         start=(t == 0), stop=(t == nj - 1))

                # local tiles run while the collective is in flight
                for t in range(njl):
                    att_tile(t)

                if l > 0:
                    # import the partner's h: select the remote ag_out half
                    ha = hgp.tile([P, NS], BF16, name=f"ha_{l}", tag="ha")
                    hb = hgp.tile([P, NS], BF16, name=f"hb_{l}", tag="hb")
                    nc.vector.tensor_scalar(ha[:], hg0[:], selt[:, 0:1],
                                            None, OP.mult)
                    nc.vector.tensor_scalar(hb[:], hg1[:], selt[:, 1:2],
                                            None, OP.mult)
                    nc.vector.tensor_tensor(hT[:, NS:N], ha[:], hb[:],
                                            OP.add)
                    prep_group(2)
                    prep_group(3)

                for t in range(njl, nj):
                    att_tile(t)

                # ---- normalize + residual (transposed layout) ----
                # 1/s in [128, 8] column shape (DVE recip is per-element
                # slow; keep free dim tiny), then broadcast r back over
                # partitions via per-tile ones-matmuls (bf16).
                s_row = smallp.tile([1, NS], F32, name=f"srow{l}",
                                    tag="srow")
                nc.vector.tensor_copy(s_row[:], psS[:])
                scol = pe_pool.tile([P, nit], F32, name=f"scol{l}",
                                    tag="pe")
                for k in range(nit):
                    nc.tensor.transpose(scol[:, k:k + 1],
                                        s_row[0:1, k * P:(k + 1) * P],
                                        identf[0:1, 0:1])
                rs = smallp.tile([P, nit], F32, name=f"rs{l}", tag="rs")
                nc.vector.reciprocal(rs[:], scol[:])
                rsb = smallp.tile([P, nit], BF16, name=f"rsb{l}", tag="rsb")
                nc.vector.tensor_copy(rsb[:], rs[:])
                r_row = smallp.tile([1, NS], BF16, name=f"rrow{l}",
                                    tag="rrow")
                for ch in range(nch):
                    rp = pe_pool.tile([1, 512], BF16, name=f"rp{l}_{ch}",
                                      tag="pe")
                    for k in range(4):
                        it = ch * 4 + k
                        nc.tensor.transpose(rp[0:1, k * P:(k + 1) * P],
                                            rsb[:, it:it + 1], identb[:])
                    nc.vector.tensor_copy(r_row[0:1,
                                                ch * 512:(ch + 1) * 512],
                                          rp[:])
                rbc = fbp.tile([P, NS], BF16, name=f"rbc{l}", tag="rbc")
                for ch in range(nch):
                    sl = slice(ch * 512, (ch + 1) * 512)
                    ps = pe_pool.tile([P, 512], F32, name=f"bcr{l}_{ch}",
                                      tag="pe")
                    nc.tensor.matmul(ps[:], onesrowb[:], r_row[0:1, sl],
                                     start=True, stop=True)
                    nc.vector.tensor_copy(rbc[:, sl], ps[:])
                tmp = qp.tile([P, NS], F32, name=f"tmp{l}", tag="tf")
                nc.vector.tensor_tensor(tmp[:], psAT[:], rbc[:], OP.mult)
                xTsb = xtp.tile([P, NS], BF16, name=f"xTsb{l + 1}",
                                tag="xTsb")
                nc.vector.tensor_tensor(xTsb[:], tmp[:], xTs[:], OP.add)
                if not last:
                    xTs_new = xtp.tile([P, NS], F32, name=f"xTs{l + 1}",
                                       tag="xTs")
                    nc.vector.tensor_tensor(xTs_new[:], tmp[:], xTs[:],
                                            OP.add)
                    xTs = xTs_new

            # ---- final linear: outT = relu(WtT^T @ xTsb + bt) ----
            for g in range(nH):
                for ch in range(nch):
                    sl = slice(ch * 512, (ch + 1) * 512)
                    ps = pe_pool.tile([P, 512], F32, name=f"ops{g}_{ch}",
                                      tag="pe")
                    nc.tensor.matmul(ps[:], WtTt[:, g * P:(g + 1) * P],
                                     xTsb[:, sl], start=True, stop=True)
                    oc = ocp.tile([P, 512], F32, name=f"oc{g}_{ch}",
                                  tag="oc")
                    nc.vector.tensor_scalar(oc[:], ps[:], btpt[:, g:g + 1],
                                            0.0, OP.add, OP.max)
                    nc.sync.dma_start(
                        out_ext.ap()[g * P:(g + 1) * P, sl], oc[:])

    if legalize:
        _legalize_waits(nc)
    return nc


def make_in_maps(x, adj, Ws, bs, avs, Wt, bt, num_cores, NS):
    """Per-core input dicts. Core c -> (graph c//2, row-half c%2).
    Column (j) layout is [local | remote] per core."""
    B, N, D = x.shape
    H = Wt.shape[0]
    nH = H // P
    x = np.ascontiguousarray(x, np.float32)
    adj = np.asarray(adj)
    shared = {"WtT": np.ascontiguousarray(
                  np.asarray(Wt, np.float32).T).astype(ml_dtypes.bfloat16),
              "btp": np.ascontiguousarray(
                  np.asarray(bt, np.float32).reshape(nH, P).T)}
    for l, (W, b, a) in enumerate(zip(Ws, bs, avs)):
        shared[f"WT{l}"] = np.ascontiguousarray(
            np.asarray(W, np.float32).T).astype(ml_dtypes.bfloat16)
        shared[f"bv{l}"] = np.ascontiguousarray(
            np.asarray(b, np.float32).reshape(D, 1))
        shared[f"av{l}"] = np.ascontiguousarray(
            np.stack([np.asarray(a, np.float32)[:D, 0],
                      np.asarray(a, np.float32)[D:, 0]],
                     axis=1)).astype(ml_dtypes.bfloat16)
    in_maps = []
    for c in range(num_cores):
        b, s = c // 2, c % 2
        m = dict(shared)
        xT = np.ascontiguousarray(x[b].T)
        loc = slice(s * NS, (s + 1) * NS)
        rem = slice((1 - s) * NS, (2 - s) * NS)
        m["xTs"] = np.ascontiguousarray(xT[:, loc])
        m["xTsb"] = m["xTs"].astype(ml_dtypes.bfloat16)
        m["xTb"] = np.concatenate([xT[:, loc], xT[:, rem]],
                                  axis=1).astype(ml_dtypes.bfloat16)
        adjT = adj[b, loc, :].T.astype(ml_dtypes.bfloat16)  # [N j, NS i]
        m["maskTb"] = np.ascontiguousarray(
            np.concatenate([adjT[loc, :], adjT[rem, :]], axis=0))
        # selector: remote half = ag_out[1-s]
        selv = np.zeros((P, 2), np.float32)
        selv[:, 1 - s] = 1.0
        m["sel"] = selv
        in_maps.append(m)
    return in_maps


_NC_CACHE = {}


def kernel(x, adj, W0, b0, W1, b1, W2, b2, a0, a1, a2, Wt, bt):
    B, N, D = 4, 2048, 128
    H = 256
    NUM_CORES = 8
    NS = N // 2
    pair_groups = [[2 * i, 2 * i + 1] for i in range(NUM_CORES // 2)]

    key = (N, NS, D, H, NUM_CORES)
    if key not in _NC_CACHE:
        _NC_CACHE[key] = build_gat_nc(N, NS, D, H, NUM_CORES, pair_groups)
    nc = _NC_CACHE[key]

    in_maps = make_in_maps(np.asarray(x), np.asarray(adj),
                           [W0, W1, W2], [b0, b1, b2], [a0, a1, a2],
                           np.asarray(Wt), np.asarray(bt), NUM_CORES, NS)
    res = run_bass_kernel_spmd(nc, in_maps, list(range(NUM_CORES))).results
    out = np.empty((B, N, H), np.float32)
    for c in range(NUM_CORES):
        b, s = c // 2, c % 2
        out[b, s * NS:(s + 1) * NS, :] = res[c]["outT_s"].T
    return out


# revision 10
# speedup vs baseline: 2.9698x; 1.2807x over previous
"""GAT (3-layer graph attention + final linear) Trainium2 Bass kernel.

Problem: B=4 graphs, N=2048 atoms, D=128, H=256.
  per layer: h = relu(x @ W.T + b); e_ij = leaky_relu(f1_i + f2_j, 0.01)
  masked by adj; att = softmax_j(e); x = x + att @ h.
  final: relu(x @ Wt.T + bt).

Sharding: 8 cores; core c -> (graph b=c//2, row-half s=c%2 of the NxN
attention). Each core computes attention for its 1024 rows (i) over all
2048 columns (j), in a core-local [local|remote] column layout: j tiles
0-7 are the core's OWN rows (h computed locally), 8-15 the partner's
(via a pair AllGather that overlaps the local tiles). The host permutes
the mask and xT inputs to match, so the program is SPMD-uniform.

Key structure (all matmuls bf16, logits fused into ACT):
  - adj transposed on the HOST into a bf16 0/1 mask, [j, i] layout,
    j-tiles permuted local-first per core.
  - logits built inside the activation pass: t = Prelu(f1bc + f2col_j,
    alpha=.01), q = Exp(t); both share one ACT table set. No row-max
    needed: logits are bounded (~36) so f32 exp is safe.
  - mask applied multiplicatively after exp on DVE (bf16).
  - aggregation transposed: psAT[d,i] += hnat_j^T @ p_j; row sums via a
    ones-column matmul into psS[1,i]. Normalize + residual stay in the
    transposed layout: 1/s computed in a [128,8] column shape (DVE
    reciprocal is ~6.4ns/elem, so never on [1,N] rows), broadcast back
    over partitions with ones-matmuls.
  - AllGather import is SPMD-uniform and runs on the idle GpSimd engine:
    remote = (hg0 + hg1) - local, exact in f32 for bf16 inputs.
  - output written transposed [H, NS]; host transposes back.
"""

import numpy as np
import ml_dtypes

import concourse.bass as bass
import concourse.mybir as mybir
import concourse.tile as tile
from concourse import masks
from concourse.bass_utils import run_bass_kernel_spmd

P = 128
F32 = mybir.dt.float32
BF16 = mybir.dt.bfloat16
AF = mybir.ActivationFunctionType
OP = mybir.AluOpType


def _legalize_waits(nc, dma_limit=1, engine_limit=1):
    """Walrus can encode only 1 sem wait on a DMA instruction and ~2 on an
    engine instruction. Move excess waits onto standalone EventSemaphore
    instructions (1 wait each) inserted just before the offender on the
    same engine."""
    counter = [0]

    def split(ins):
        si = ins.sync_info
        if si is None:
            return None
        limit = dma_limit if type(ins).__name__.startswith("InstDMA") \
            else engine_limit
        waits = list(si.on_wait)
        if len(waits) <= limit:
            return None
        keep = waits[-limit:] if limit > 0 else []
        extra = waits[:-limit] if limit > 0 else waits
        evs = []
        for w in extra:
            counter[0] += 1
            evs.append(mybir.InstEventSemaphore(
                name=f"evsplit{counter[0]}", engine=ins.engine,
                sync_info=mybir.SyncInfo(on_wait=[w], on_update=[])))
        ins.sync_info = mybir.SyncInfo(on_wait=keep,
                                       on_update=list(si.on_update))
        return evs

    for f in nc.m.functions:
        for blk in f.blocks:
            new_list = []
            changed = False
            for ins in blk.instructions:
                evs = split(ins)
                if evs:
                    new_list.extend(evs)
                    changed = True
                new_list.append(ins)
            if changed:
                blk.instructions = new_list


def build_gat_nc(N, NS, D, H, num_cores, pair_groups, nlayers=3,
                 legalize=True):
    assert D == P and NS % 512 == 0 and N % 512 == 0
    nj = N // P        # j tiles (16)
    njl = nj // 2      # local j tiles (8)
    nch = NS // 512    # 512-chunks in the i shard (2)
    nit = NS // P      # i tiles (8)
    nH = H // P

    nc = bass.Bass("TRN2", target_bir_lowering=False, debug=False,
                   num_devices=num_cores)

    # ---- I/O ----
    xTsb_in = nc.dram_tensor("xTsb", [P, NS], BF16, kind="ExternalInput")
    xTs_in = nc.dram_tensor("xTs", [P, NS], F32, kind="ExternalInput")
    xTb_in = nc.dram_tensor("xTb", [P, N], BF16, kind="ExternalInput")
    mask_in = nc.dram_tensor("maskTb", [N, NS], BF16, kind="ExternalInput")
    WT_in = [nc.dram_tensor(f"WT{l}", [D, D], BF16, kind="ExternalInput")
             for l in range(nlayers)]
    bv_in = [nc.dram_tensor(f"bv{l}", [D, 1], F32, kind="ExternalInput")
             for l in range(nlayers)]
    av_in = [nc.dram_tensor(f"av{l}", [D, 2], BF16, kind="ExternalInput")
             for l in range(nlayers)]
    WtT_in = nc.dram_tensor("WtT", [D, H], BF16, kind="ExternalInput")
    btp_in = nc.dram_tensor("btp", [P, nH], F32, kind="ExternalInput")
    out_ext = nc.dram_tensor("outT_s", [H, NS], F32, kind="ExternalOutput")

    # DRAM bounce buffers for the pair AllGather of h shards (layers 1..)
    ag_in = [None] + [nc.dram_tensor(f"ag_in{l}", [P, NS], BF16)
                      for l in range(1, nlayers)]
    ag_out = [None] + [nc.dram_tensor(f"ag_out{l}", [2 * P, NS], BF16)
                       for l in range(1, nlayers)]

    with tile.TileContext(nc) as tc:
        import contextlib
        ctx = contextlib.ExitStack()
        with ctx:
            persist = ctx.enter_context(tc.tile_pool(name="persist", bufs=1))
            htp = ctx.enter_context(tc.tile_pool(name="htp", bufs=2))
            hgp = ctx.enter_context(tc.tile_pool(name="hgp", bufs=2))
            xtp = ctx.enter_context(tc.tile_pool(name="xtp", bufs=2))
            fbp = ctx.enter_context(tc.tile_pool(name="fbp", bufs=2))
            qp = ctx.enter_context(tc.tile_pool(name="qp", bufs=2))
            hnp = ctx.enter_context(tc.tile_pool(name="hnp", bufs=2))
            smallp = ctx.enter_context(tc.tile_pool(name="smallp", bufs=2))
            ocp = ctx.enter_context(tc.tile_pool(name="ocp", bufs=2))
            pe_pool = ctx.enter_context(
                tc.tile_pool(name="pe_pool", bufs=4, space="PSUM"))
            attp = ctx.enter_context(
                tc.tile_pool(name="attp", bufs=1, space="PSUM"))
            spp = ctx.enter_context(
                tc.tile_pool(name="spp", bufs=1, space="PSUM"))

            identb = persist.tile([P, P], BF16)
            masks.make_identity(nc, identb[:])
            identf = persist.tile([P, P], F32)
            masks.make_identity(nc, identf[:])
            onescol = persist.tile([P, 1], BF16)
            nc.vector.memset(onescol[:], 1.0)
            onesrowb = persist.tile([1, P], BF16)
            nc.vector.memset(onesrowb[:], 1.0)

            # ---- weights (critical-path inputs first) ----
            WT = [persist.tile([D, D], BF16, name=f"WT{l}", tag=f"WT{l}")
                  for l in range(nlayers)]
            bv = [persist.tile([D, 1], F32, name=f"bv{l}", tag=f"bv{l}")
                  for l in range(nlayers)]
            av = [persist.tile([D, 2], BF16, name=f"av{l}", tag=f"av{l}")
                  for l in range(nlayers)]
            WtTt = persist.tile([D, H], BF16)
            btpt = persist.tile([P, nH], F32)
            nc.sync.dma_start(WT[0][:], WT_in[0].ap())
            nc.sync.dma_start(bv[0][:], bv_in[0].ap())
            nc.sync.dma_start(av[0][:], av_in[0].ap())
            xTb = persist.tile([P, N], BF16)
            nc.sync.dma_start(xTb[:, 0:NS], xTb_in.ap()[:, 0:NS])
            nc.sync.dma_start(xTb[:, NS:N], xTb_in.ap()[:, NS:N])
            for l in range(1, nlayers):
                nc.sync.dma_start(WT[l][:], WT_in[l].ap())
                nc.sync.dma_start(bv[l][:], bv_in[l].ap())
                nc.sync.dma_start(av[l][:], av_in[l].ap())
            nc.sync.dma_start(WtTt[:], WtT_in.ap())
            nc.sync.dma_start(btpt[:], btp_in.ap())

            # ---- initial x state (transposed bf16 + f32 residual) ----
            xTsb = xtp.tile([P, NS], BF16, name="xTsb0", tag="xTsb")
            nc.sync.dma_start(xTsb[:], xTsb_in.ap())
            xTs = xtp.tile([P, NS], F32, name="xTs0", tag="xTs")
            nc.sync.dma_start(xTs[:], xTs_in.ap())

            # ---- adjacency mask tiles (bf16 0/1, [j, i], local-first) ----
            maskM = [persist.tile([P, NS], BF16, name=f"maskM{j}",
                                  tag=f"maskM{j}") for j in range(nj)]
            for j in range(nj):
                nc.sync.dma_start(maskM[j][:],
                                  mask_in.ap()[j * P:(j + 1) * P, :])

            # deferred off-critical-path emission (residual f32 add)
            pending = []

            for l in range(nlayers):
                last = l == nlayers - 1
                hT = htp.tile([P, N], BF16, name=f"hT{l}", tag="hT")
                # -- local h -> hT[:, 0:NS] --
                loc_src = xTb if l == 0 else xTsb
                for ch in range(nch):
                    sl = slice(ch * 512, (ch + 1) * 512)
                    ps = pe_pool.tile([P, 512], F32, name=f"hps{l}_{ch}",
                                      tag="pe")
                    nc.tensor.matmul(ps[:], WT[l][:], loc_src[:, sl],
                                     start=True, stop=True)
                    nc.vector.tensor_scalar(hT[:, sl], ps[:], bv[l][:],
                                            0.0, OP.add, OP.max)
                if l > 0:
                    nc.gpsimd.dma_start(ag_in[l].ap(), hT[:, 0:NS])
                    nc.gpsimd.collective_compute(
                        "AllGather", OP.bypass, replica_groups=pair_groups,
                        ins=[ag_in[l].ap()], outs=[ag_out[l].ap()])
                    # deferred residual f32 add rides in the collective
                    # shadow on the gpsimd queue
                    for fn in pending:
                        fn()
                    pending = []
                    hg0 = hgp.tile([P, NS], BF16, name=f"hg0_{l}",
                                   tag="hg0")
                    hg1 = hgp.tile([P, NS], BF16, name=f"hg1_{l}",
                                   tag="hg1")
                    nc.gpsimd.dma_start(hg0[:], ag_out[l].ap()[0:P, :])
                    nc.gpsimd.dma_start(hg1[:], ag_out[l].ap()[P:2 * P, :])
                    # remote = (hg0 + hg1) - local  (exact in f32)
                    hsum = hgp.tile([P, NS], F32, name=f"hsum{l}",
                                    tag="hsum")
                    nc.gpsimd.tensor_tensor(hsum[:], hg0[:], hg1[:], OP.add)
                    nc.gpsimd.tensor_tensor(hT[:, NS:N], hsum[:],
                                            hT[:, 0:NS], OP.subtract)

                # -- f1 over shard rows (from local h half), bf16 --
                f1row = smallp.tile([1, NS], BF16, name=f"f1row{l}",
                                    tag="f1row")
                for ch in range(nch):
                    sl = slice(ch * 512, (ch + 1) * 512)
                    psf = pe_pool.tile([2, 512], F32, name=f"fps{l}_{ch}",
                                       tag="pe")
                    nc.tensor.matmul(psf[:], av[l][:], hT[:, sl],
                                     start=True, stop=True)
                    nc.vector.tensor_copy(f1row[0:1, sl], psf[0:1, :])
                f1bc = fbp.tile([P, NS], BF16, name=f"f1bc{l}", tag="f1bc")
                for ch in range(nch):
                    sl = slice(ch * 512, (ch + 1) * 512)
                    ps = pe_pool.tile([P, 512], F32, name=f"bcf{l}_{ch}",
                                      tag="pe")
                    nc.tensor.matmul(ps[:], onesrowb[:], f1row[0:1, sl],
                                     start=True, stop=True)
                    nc.vector.tensor_copy(f1bc[:, sl], ps[:])

                # -- remote h for layer 0 (computed locally) --
                if l == 0:
                    for ch in range(nch):
                        sl = slice(NS + ch * 512, NS + (ch + 1) * 512)
                        ps = pe_pool.tile([P, 512], F32,
                                          name=f"hrps{l}_{ch}", tag="pe")
                        nc.tensor.matmul(ps[:], WT[l][:], xTb[:, sl],
                                         start=True, stop=True)
                        nc.vector.tensor_scalar(hT[:, sl], ps[:], bv[l][:],
                                                0.0, OP.add, OP.max)

                # -- per-j-tile [f1col, f2col] and natural-layout h --
                f2c = [None] * (nj // 4)
                hnatg = [None] * (nj // 4)

                def prep_group(g, l=l, hT=hT, f2c=f2c, hnatg=hnatg):
                    psc = pe_pool.tile([P, 8], F32, name=f"psc{l}_{g}",
                                       tag="pe")
                    for q in range(4):
                        t = g * 4 + q
                        nc.tensor.matmul(psc[:, 2 * q:2 * q + 2],
                                         hT[:, t * P:(t + 1) * P], av[l][:],
                                         start=True, stop=True)
                    fc = smallp.tile([P, 8], F32, name=f"f2c{l}_{g}",
                                     tag=f"f2c{g}")
                    nc.vector.tensor_copy(fc[:], psc[:])
                    f2c[g] = fc
                    pst = pe_pool.tile([P, 512], BF16, name=f"htp{l}_{g}",
                                       tag="pe")
                    for q in range(4):
                        t = g * 4 + q
                        nc.tensor.transpose(pst[:, q * P:(q + 1) * P],
                                            hT[:, t * P:(t + 1) * P],
                                            identb[:])
                    hn = hnp.tile([P, 512], BF16, name=f"hng{l}_{g}",
                                  tag=f"hng{g}")
                    nc.vector.tensor_copy(hn[:], pst[:])
                    hnatg[g] = hn

                prep_group(0)
                prep_group(1)
                if l == 0:
                    prep_group(2)
                    prep_group(3)

                # ---- attention: logits on ACT, mask on DVE, agg on PE ----
                psAT = attp.tile([P, NS], F32, name=f"psAT{l}", tag="att")
                psS = spp.tile([1, NS], F32, name=f"psS{l}", tag="s")

                def att_tile(t, l=l, psAT=psAT, psS=psS, f2c=f2c,
                             hnatg=hnatg, f1bc=f1bc):
                    g, q = t // 4, t % 4
                    tf = qp.tile([P, NS], F32, name=f"tf{l}_{t}", tag="tf")
                    nc.scalar.activation(tf[:], f1bc[:], AF.Prelu,
                                         bias=f2c[g][:, 2 * q + 1:2 * q + 2],
                                         scale=1.0, alpha=0.01)
                    qb = qp.tile([P, NS], BF16, name=f"qb{l}_{t}", tag="qb")
                    nc.scalar.activation(qb[:], tf[:], AF.Exp)
                    pb = qp.tile([P, NS], BF16, name=f"pb{l}_{t}", tag="pb")
                    nc.vector.tensor_tensor(pb[:], qb[:], maskM[t][:],
                                            OP.mult)
                    # on the last tile close the row-sum bank first so the
                    # reciprocal chain can start before the last agg matmul
                    mm = []
                    for ch in range(nch):
                        sl = slice(ch * 512, (ch + 1) * 512)
                        mm.append((psAT, hnatg[g][:, q * P:(q + 1) * P],
                                   pb[:, sl], sl, False))
                        mm.append((psS, onescol[:], pb[:, sl], sl, True))
                    if t == nj - 1:
                        mm.sort(key=lambda x: not x[4])
                    for dst, st, mv, sl, is_s in mm:
                        if is_s:
                            nc.tensor.matmul(psS[0:1, sl], st, mv,
                                             start=(t == 0),
                                             stop=(t == nj - 1))
                        else:
                            nc.tensor.matmul(psAT[:, sl], st, mv,
                                             start=(t == 0),
                                             stop=(t == nj - 1))

                for t in range(njl):
                    att_tile(t)
                if l > 0:
                    prep_group(2)
                    prep_group(3)
                for t in range(njl, nj):
                    att_tile(t)

                # ---- normalize + residual (transposed layout) ----
                s_row = smallp.tile([1, NS], F32, name=f"srow{l}",
                                    tag="srow")
                nc.vector.tensor_copy(s_row[:], psS[:])
                scol = pe_pool.tile([P, nit], F32, name=f"scol{l}",
                                    tag="pe")
                for k in range(nit):
                    nc.tensor.transpose(scol[:, k:k + 1],
                                        s_row[0:1, k * P:(k + 1) * P],
                                        identf[0:1, 0:1])
                rs = smallp.tile([P, nit], F32, name=f"rs{l}", tag="rs")
                nc.vector.reciprocal(rs[:], scol[:])
                rsb = smallp.tile([P, nit], BF16, name=f"rsb{l}", tag="rsb")
                nc.vector.tensor_copy(rsb[:], rs[:])
                r_row = smallp.tile([1, NS], BF16, name=f"rrow{l}",
                                    tag="rrow")
                for ch in range(nch):
                    rp = pe_pool.tile([1, 512], BF16, name=f"rp{l}_{ch}",
                                      tag="pe")
                    for k in range(4):
                        it = ch * 4 + k
                        nc.tensor.transpose(rp[0:1, k * P:(k + 1) * P],
                                            rsb[:, it:it + 1], identb[:])
                    nc.vector.tensor_copy(r_row[0:1,
                                                ch * 512:(ch + 1) * 512],
                                          rp[:])
                xTsb_new = xtp.tile([P, NS], BF16, name=f"xTsb{l + 1}",
                                    tag="xTsb")
                tmps = []
                for ch in range(nch):
                    sl = slice(ch * 512, (ch + 1) * 512)
                    bps = pe_pool.tile([P, 512], F32, name=f"bcr{l}_{ch}",
                                       tag="pe")
                    nc.tensor.matmul(bps[:], onesrowb[:], r_row[0:1, sl],
                                     start=True, stop=True)
                    rbc = smallp.tile([P, 512], BF16, name=f"rbc{l}_{ch}",
                                      tag=f"rbc{ch}")
                    nc.vector.tensor_copy(rbc[:], bps[:])
                    tmp = qp.tile([P, 512], F32, name=f"tmp{l}_{ch}",
                                  tag=f"tmp{ch}")
                    nc.vector.tensor_tensor(tmp[:], psAT[:, sl], rbc[:],
                                            OP.mult)
                    nc.vector.tensor_tensor(xTsb_new[:, sl], tmp[:],
                                            xTs[:, sl], OP.add)
                    tmps.append(tmp)
                if not last:
                    xTs_new = xtp.tile([P, NS], F32, name=f"xTs{l + 1}",
                                       tag="xTs")

                    def resid(xTs_new=xTs_new, tmps=tmps, xTs=xTs):
                        for ch in range(nch):
                            sl = slice(ch * 512, (ch + 1) * 512)
                            nc.gpsimd.tensor_tensor(xTs_new[:, sl],
                                                    tmps[ch][:],
                                                    xTs[:, sl], OP.add)
                    pending.append(resid)
                    xTs = xTs_new
                xTsb = xTsb_new

            # ---- final linear: outT = relu(WtT^T @ xTsb + bt) ----
            for g in range(nH):
                for ch in range(nch):
                    sl = slice(ch * 512, (ch + 1) * 512)
                    ps = pe_pool.tile([P, 512], F32, name=f"ops{g}_{ch}",
                                      tag="pe")
                    nc.tensor.matmul(ps[:], WtTt[:, g * P:(g + 1) * P],
                                     xTsb[:, sl], start=True, stop=True)
                    oc = ocp.tile([P, 512], F32, name=f"oc{g}_{ch}",
                                  tag="oc")
                    nc.vector.tensor_scalar(oc[:], ps[:], btpt[:, g:g + 1],
                                            0.0, OP.add, OP.max)
                    nc.sync.dma_start(
                        out_ext.ap()[g * P:(g + 1) * P, sl], oc[:])

    if legalize:
        _legalize_waits(nc)
    return nc


def make_in_maps(x, adj, Ws, bs, avs, Wt, bt, num_cores, NS):
    """Per-core input dicts. Core c -> (graph c//2, row-half c%2).
    Column (j) layout is [local | remote] per core."""
    B, N, D = x.shape
    H = Wt.shape[0]
    nH = H // P
    x = np.ascontiguousarray(x, np.float32)
    adj = np.asarray(adj)
    shared = {"WtT": np.ascontiguousarray(
                  np.asarray(Wt, np.float32).T).astype(ml_dtypes.bfloat16),
              "btp": np.ascontiguousarray(
                  np.asarray(bt, np.float32).reshape(nH, P).T)}
    for l, (W, b, a) in enumerate(zip(Ws, bs, avs)):
        shared[f"WT{l}"] = np.ascontiguousarray(
            np.asarray(W, np.float32).T).astype(ml_dtypes.bfloat16)
        shared[f"bv{l}"] = np.ascontiguousarray(
            np.asarray(b, np.float32).reshape(D, 1))
        shared[f"av{l}"] = np.ascontiguousarray(
            np.stack([np.asarray(a, np.float32)[:D, 0],
                      np.asarray(a, np.float32)[D:, 0]],
                     axis=1)).astype(ml_dtypes.bfloat16)
    in_maps = []
    for c in range(num_cores):
        b, s = c // 2, c % 2
        m = dict(shared)
        xT = np.ascontiguousarray(x[b].T)
        loc = slice(s * NS, (s + 1) * NS)
        rem = slice((1 - s) * NS, (2 - s) * NS)
        m["xTs"] = np.ascontiguousarray(xT[:, loc])
        m["xTsb"] = m["xTs"].astype(ml_dtypes.bfloat16)
        m["xTb"] = np.concatenate([xT[:, loc], xT[:, rem]],
                                  axis=1).astype(ml_dtypes.bfloat16)
        adjT = adj[b, loc, :].T.astype(ml_dtypes.bfloat16)  # [N j, NS i]
        m["maskTb"] = np.ascontiguousarray(
            np.concatenate([adjT[loc, :], adjT[rem, :]], axis=0))
        in_maps.append(m)
    return in_maps


_NC_CACHE = {}


def kernel(x, adj, W0, b0, W1, b1, W2, b2, a0, a1, a2, Wt, bt):
    B, N, D = 4, 2048, 128
    H = 256
    NUM_CORES = 8
    NS = N // 2
    pair_groups = [[2 * i, 2 * i + 1] for i in range(NUM_CORES // 2)]

    key = (N, NS, D, H, NUM_CORES)
    if key not in _NC_CACHE:
        _NC_CACHE[key] = build_gat_nc(N, NS, D, H, NUM_CORES, pair_groups)
    nc = _NC_CACHE[key]

    in_maps = make_in_maps(np.asarray(x), np.asarray(adj),
                           [W0, W1, W2], [b0, b1, b2], [a0, a1, a2],
                           np.asarray(Wt), np.asarray(bt), NUM_CORES, NS)
    res = run_bass_kernel_spmd(nc, in_maps, list(range(NUM_CORES))).results
    out = np.empty((B, N, H), np.float32)
    for c in range(NUM_CORES):
        b, s = c // 2, c % 2
        out[b, s * NS:(s + 1) * NS, :] = res[c]["outT_s"].T
    return out
